# revision 9
# baseline (speedup 1.0000x reference)
"""GQA kernel for Trainium2, sharded over 8 NeuronCores.

Problem: B=2, S=2048, H=2048, NH=16 q-heads, KVH=4 kv-heads, D=128.
Sharding: core c -> (batch b = c//4, kv-head k = c%4). Each core computes the
full attention for its 4 query heads + its kv head on its batch, plus the
row-parallel partial of the output projection. Host sums the 4 partials per
batch and adds the output bias.

v3 schedule:
  - chunk-pipelined: per q-chunk c: projection -> attention (4 heads) ->
    output projection, with chunk c-1's o-proj tiles interleaved between
    chunk c's attention blocks so the PE queue always has filler work.
  - batched DMA: weights/hidden loaded as a few big multi-tile transfers,
    split across engine queues (sync=weights/out, gpsimd=hidden, vector=
    consts) so startup loads run in parallel.
  - causal handling: diagonal-block score/exp/AV restricted to the
    unmasked column range (512-128*dq); single 128x128 triangular -1e4
    mask applied via an accumulated identity matmul.
  - softmax denominator: DVE bf16 adds -> ones-matmul reduce ->
    reciprocal_approx_fast -> gpsimd partition_broadcast -> one DVE mul.
  - o-proj bias on host; bf16 output partials.
"""

import numpy as np
import ml_dtypes

import concourse.bass as bass
import concourse.mybir as mybir
import concourse.tile as tile
from concourse import bacc

BF16 = ml_dtypes.bfloat16
F32 = mybir.dt.float32
BF = mybir.dt.bfloat16

B, S, H = 2, 2048, 2048
NH, KVH, D = 16, 4, 128
G = NH // KVH  # q heads per kv head / per core
N_CORES = 8
SCALE = 1.0 / float(np.sqrt(D))
NEG = -10000.0

SQ = 512              # q-chunk (psum free width)
NQC = S // SQ         # 4 q chunks
NKT = S // 128        # 16 kv tiles / token tiles
NHT = H // 128        # 16 hidden k-tiles
ROWS = G + 2          # 6 projection row-blocks: 4 q heads, k, v


def build_nc(num_devices: int = N_CORES) -> bass.Bass:
    nc = bacc.Bacc("TRN2", num_devices=num_devices)

    hT = nc.dram_tensor("hT", [NQC, H * SQ], BF, kind="ExternalInput").ap()
    wqkvT = nc.dram_tensor("wqkvT", [ROWS, H * 128], BF, kind="ExternalInput").ap()
    bqkv = nc.dram_tensor("bqkv", [1, ROWS * 128], BF, kind="ExternalInput").ap()
    cosT = nc.dram_tensor("cosT", [128, S], BF, kind="ExternalInput").ap()
    sinTs = nc.dram_tensor("sinTs", [128, S], BF, kind="ExternalInput").ap()
    rotT = nc.dram_tensor("rotT", [128, 128], BF, kind="ExternalInput").ap()
    mask128 = nc.dram_tensor("mask128", [128, 128], BF, kind="ExternalInput").ap()
    woT = nc.dram_tensor("woT", [G * 128, H], BF, kind="ExternalInput").ap()
    id128 = nc.dram_tensor("id128", [128, 128], BF, kind="ExternalInput").ap()
    out = nc.dram_tensor("out", [S, H], BF, kind="ExternalOutput").ap()

    with tile.TileContext(nc) as tc:
        with (
            tc.tile_pool(name="consts", bufs=1) as consts,
            tc.tile_pool(name="persist", bufs=1) as persist,
            tc.tile_pool(name="hp", bufs=2) as hp,
            tc.tile_pool(name="work", bufs=3) as work,
            tc.tile_pool(name="ps", bufs=1, space="PSUM") as ps,
        ):
            # ---- weights: 4 quarter-loads on the sync queue ----
            wq_all = persist.tile([128, NHT * ROWS * 128], BF, tag="wq", name="wq_all")
            for i, m in enumerate([G, G + 1] + list(range(G))):
                nc.sync.dma_start(
                    out=wq_all[:, m * NHT * 128:(m + 1) * NHT * 128],
                    in_=wqkvT[i:i + 1, :].rearrange("o (p w) -> p (o w)", p=128),
                )


            def wq_sl(kt, m):
                return wq_all[:, m * NHT * 128 + kt * 128: m * NHT * 128 + (kt + 1) * 128]

            # ---- constants (scalar queue), bias first ----
            bias_sb = consts.tile([1, ROWS * 128], BF, tag="bias", name="bias")
            nc.scalar.dma_start(out=bias_sb, in_=bqkv)
            cos_sb = consts.tile([128, S], BF, tag="cos", name="cos")
            nc.scalar.dma_start(out=cos_sb, in_=cosT)
            sin_sb = consts.tile([128, S], BF, tag="sin", name="sin")
            nc.scalar.dma_start(out=sin_sb, in_=sinTs)
            mask_sb = consts.tile([128, 128], BF, tag="mask", name="mask")
            nc.scalar.dma_start(out=mask_sb, in_=mask128)
            id_sb = consts.tile([128, 128], BF, tag="id", name="id")
            nc.scalar.dma_start(out=id_sb, in_=id128)
            rt_sb = consts.tile([128, 128], BF, tag="rt", name="rt")
            nc.scalar.dma_start(out=rt_sb, in_=rotT)
            wo_all = persist.tile([128, G * H], BF, tag="wo", name="wo_all")
            ones_col = consts.tile([128, 1], BF, tag="ones_col", name="ones_col")
            nc.vector.memset(ones_col, 1.0)
            ones_row = consts.tile([1, SQ], BF, tag="ones_row", name="ones_row")
            nc.vector.memset(ones_row, 1.0)
            scr_bf = consts.tile([128, SQ], BF, tag="scr", name="scr_bf")
            nc.vector.memset(scr_bf, 0.125)
            warm = ps.tile([128, SQ], F32, tag="aux", bufs=3, name="warm")
            for _ in range(16):
                nc.tensor.matmul(warm[0:1, :], ones_col, scr_bf, start=True, stop=True)

            # persistent activations
            qk_sb = [persist.tile([128, S], BF, tag=f"qk{m}", name=f"qk{m}") for m in range(G + 1)]
            vT_sb = persist.tile([128, S], BF, tag="vT", name="vT")
            v_sb = [persist.tile([128, 128], BF, tag=f"v{j}", name=f"v{j}") for j in range(NKT)]
            xT_sb = [persist.tile([128, S], BF, tag=f"xT{h}", name=f"xT{h}") for h in range(G)]
            kT = qk_sb[G]

            dmaq = [nc.sync, nc.gpsimd, nc.scalar]
            oproj_ctr = [0]

            def oproj(t):
                """output projection for token tile t."""
                ts_ = slice(t * 128, (t + 1) * 128)
                o_row = work.tile([128, H], BF, tag="o_row", bufs=2, name="o_row")
                for n in range(NQC):
                    ns = slice(n * SQ, (n + 1) * SQ)
                    op = ps.tile([128, SQ], F32, tag="aux", bufs=3, name="op")
                    for g in range(G):
                        nc.tensor.matmul(
                            op, xT_sb[g][:, ts_], wo_all[:, g * H + n * SQ: g * H + (n + 1) * SQ],
                            start=(g == 0), stop=(g == G - 1),
                        )
                    if n % 2 == 0:
                        nc.scalar.copy(o_row[:, ns], op)
                    else:
                        nc.vector.tensor_copy(o_row[:, ns], op)
                    if n % 2 == 1:
                        q = dmaq[oproj_ctr[0] % 3]
                        oproj_ctr[0] += 1
                        q.dma_start(
                            out=out[ts_, (n - 1) * SQ:(n + 1) * SQ],
                            in_=o_row[:, (n - 1) * SQ:(n + 1) * SQ],
                        )

            def load_h(c):
                h_all = hp.tile([128, NHT * SQ], BF, tag="hall", name="h_all")
                hc = hT[c:c + 1, :].rearrange("o (p w) -> p (o w)", p=128)
                for qtr in range(8):
                    nc.gpsimd.dma_start(
                        out=h_all[:, qtr * 2 * SQ:(qtr + 1) * 2 * SQ],
                        in_=hc[:, qtr * 2 * SQ:(qtr + 1) * 2 * SQ],
                    )
                return h_all

            h_next = load_h(0)
            for c in range(NQC):
                cs = slice(c * SQ, (c + 1) * SQ)
                h_all = h_next

                # ---- projection + RoPE for chunk c ----
                # row order: k (G), v (G+1) first so attention can start asap
                for m in [G, G + 1] + list(range(G)):
                    mp = ps.tile([128, SQ], F32, tag="mm", bufs=3, name="mp")
                    for kt in range(NHT):
                        nc.tensor.matmul(
                            mp, wq_sl(kt, m),
                            h_all[:, kt * SQ:(kt + 1) * SQ],
                            start=(kt == 0), stop=False,
                        )
                    nc.tensor.matmul(
                        mp, bias_sb[:, m * 128:(m + 1) * 128], ones_row,
                        start=False, stop=True,
                    )
                    if m == G + 1:
                        # v: cast to vT, then produce v blocks for this chunk
                        nc.vector.tensor_copy(vT_sb[:, cs], mp)
                        for jj in range(4 * c, 4 * c + 4):
                            vp = ps.tile([128, SQ], F32, tag="aux", bufs=3, name="vp")
                            nc.tensor.matmul(
                                vp[:, :128], vT_sb[:, jj * 128:(jj + 1) * 128],
                                id_sb, start=True, stop=True,
                            )
                            nc.vector.tensor_copy(v_sb[jj], vp[:, :128])
                    else:
                        # q head or k: RoPE
                        tmp = work.tile([128, SQ], BF, tag="tmp", name="tmp")
                        nc.vector.tensor_copy(tmp, mp)
                        rp = ps.tile([128, SQ], F32, tag="aux", bufs=3, name="rp")
                        nc.tensor.matmul(rp, rt_sb, tmp, start=True, stop=True)
                        rots = work.tile([128, SQ], BF, tag="rots", name="rots")
                        nc.vector.tensor_mul(rots, rp, sin_sb[:, cs])
                        t1 = work.tile([128, SQ], BF, tag="t1", name="t1")
                        nc.vector.tensor_mul(t1, tmp, cos_sb[:, cs])
                        nc.vector.tensor_add(qk_sb[m][:, cs], t1, rots)

                if c + 1 < NQC:
                    h_next = load_h(c + 1)
                if c == 0:
                    nc.scalar.dma_start(
                        out=wo_all, in_=woT.rearrange("(g p) n -> p g n", p=128)
                    )

                # ---- attention for chunk c (4 heads), interleaving chunk
                # c-1's output projection between heads ----
                njt = 4 * c + 4
                for h in range(G):
                    av = ps.tile([128, SQ], F32, tag="av", bufs=2, name="av")
                    dacc = work.tile([128, SQ], BF, tag="dacc", bufs=2, name="dacc")
                    for j in range(njt):
                        dq = j - 4 * c
                        lo = max(dq, 0) * 128  # first unmasked column
                        sc = ps.tile([128, SQ], F32, tag="mm", bufs=3, name="sc")
                        nc.tensor.matmul(
                            sc[:, lo:], kT[:, j * 128:(j + 1) * 128],
                            qk_sb[h][:, c * SQ + lo:(c + 1) * SQ],
                            start=True, stop=(dq < 0),
                        )
                        if dq >= 0:
                            nc.tensor.matmul(
                                sc[:, lo:lo + 128], id_sb, mask_sb,
                                start=False, stop=True, skip_group_check=True,
                            )
                        ex = work.tile([128, SQ], BF, tag="ex", bufs=3, name="ex")
                        nc.scalar.activation(
                            ex[:, lo:], sc[:, lo:],
                            mybir.ActivationFunctionType.Exp, scale=SCALE,
                        )
                        if j == 0:
                            nc.vector.tensor_copy(dacc, ex)
                        else:
                            nc.vector.tensor_add(
                                dacc[:, lo:], dacc[:, lo:], ex[:, lo:]
                            )
                        nc.tensor.matmul(
                            av[:, lo:], v_sb[j], ex[:, lo:],
                            start=(j == 0), stop=(j == njt - 1),
                            skip_group_check=True,
                        )
                    dn = ps.tile([128, SQ], F32, tag="aux", bufs=3, name="dn")
                    nc.tensor.matmul(dn[0:1, :], ones_col, dacc, start=True, stop=True)
                    rd = work.tile([1, SQ], F32, tag="rd", bufs=2, name="rd")
                    nc.vector.reciprocal_approx_fast(rd, dn[0:1, :])
                    rdb = work.tile([1, SQ], BF, tag="rdb", bufs=2, name="rdb")
                    nc.vector.tensor_copy(rdb, rd)
                    bc = ps.tile([128, SQ], F32, tag="aux", bufs=3, name="bc")
                    nc.tensor.matmul(bc, ones_row[:, :128], rdb, start=True, stop=True)
                    bcs = work.tile([128, SQ], BF, tag="bcs", bufs=2, name="bcs")
                    nc.scalar.copy(bcs, bc)
                    nc.vector.tensor_mul(xT_sb[h][:, cs], av, bcs)
                    if c > 0:
                        oproj(4 * (c - 1) + h)

            for t in range(12, 16):
                oproj(t)
    nc.compile()
    return nc


def make_in_maps(hidden_states, cos, sin, Wq, bq, Wk, bk, Wv, bv, Wo, bo):
    """Host-side shard/pack. Returns list of 8 input dicts."""
    f32 = np.float32
    cosT = np.ascontiguousarray(np.asarray(cos).T).astype(BF16)
    sinTs = np.ascontiguousarray(np.asarray(sin).T).astype(BF16)
    R = np.zeros((128, 128), f32)
    for d in range(64):
        R[d, d + 64] = -1.0
        R[d + 64, d] = 1.0
    rotT = np.ascontiguousarray(R.T).astype(BF16)
    # triangular -1e4 bias for the in-tile diagonal: unmasked iff q >= kv
    p = np.arange(128)[:, None]
    f = np.arange(128)[None, :]
    mask128 = np.where(f >= p, 0.0, NEG).astype(BF16)
    id128 = np.eye(128, dtype=BF16)

    in_maps = []
    for core in range(N_CORES):
        b, k = core // 4, core % 4
        hTr = np.asarray(hidden_states[b]).T.reshape(NHT, 128, NQC, SQ)
        hT = np.ascontiguousarray(
            hTr.transpose(2, 1, 0, 3).reshape(NQC, 128 * NHT * SQ)
        ).astype(BF16)  # [c][p][kt][q]
        wq = Wq[512 * k:512 * (k + 1)]            # [512, H]
        wk = Wk[128 * k:128 * (k + 1)]            # [128, H]
        wv = Wv[128 * k:128 * (k + 1)]
        wcat = np.concatenate(
            [wk, wv, wq], axis=0
        ).reshape(ROWS, 128, NHT, 128)             # [m'][r][kt][p]
        wqkvT = np.ascontiguousarray(
            wcat.transpose(0, 3, 2, 1).reshape(ROWS, 128 * NHT * 128)
        ).astype(BF16)                             # [m'][p][kt][r]
        bqkv = np.concatenate(
            [bq[512 * k:512 * (k + 1)], bk[128 * k:128 * (k + 1)],
             bv[128 * k:128 * (k + 1)]]
        ).astype(BF16).reshape(1, ROWS * 128)
        woT = np.ascontiguousarray(Wo[:, 512 * k:512 * (k + 1)].T).astype(BF16)
        in_maps.append({
            "hT": hT, "wqkvT": wqkvT, "bqkv": bqkv,
            "cosT": cosT, "sinTs": sinTs, "mask128": mask128, "rotT": rotT,
            "woT": woT, "id128": id128,
        })
    return in_maps


_NC = None


def kernel(**inputs) -> np.ndarray:
    global _NC
    from concourse.bass_utils import run_bass_kernel_spmd

    if _NC is None:
        _NC = build_nc()
    in_maps = make_in_maps(**inputs)
    res = run_bass_kernel_spmd(_NC, in_maps, core_ids=list(range(N_CORES)))
    bo = np.asarray(inputs["bo"], np.float32)
    out = np.zeros((B, S, H), np.float32)
    for core in range(N_CORES):
        out[core // 4] += np.asarray(res.results[core]["out"], np.float32)
    out += bo[None, None, :]
    return out


# revision 10
# speedup vs baseline: 1.0537x; 1.0537x over previous
"""GQA kernel for Trainium2, sharded over 8 NeuronCores.

Problem: B=2, S=2048, H=2048, NH=16 q-heads, KVH=4 kv-heads, D=128.
Sharding: core c -> (batch b = c//4, kv-head k = c%4). Each core computes the
full attention for its 4 query heads + its kv head on its batch, plus the
row-parallel partial of the output projection. Host sums the 4 partials per
batch and adds the output bias.

v3 schedule:
  - chunk-pipelined: per q-chunk c: projection -> attention (4 heads) ->
    output projection, with chunk c-1's o-proj tiles interleaved between
    chunk c's attention blocks so the PE queue always has filler work.
  - batched DMA: weights/hidden loaded as a few big multi-tile transfers,
    split across engine queues (sync=weights/out, gpsimd=hidden, vector=
    consts) so startup loads run in parallel.
  - causal handling: diagonal-block score/exp/AV restricted to the
    unmasked column range (512-128*dq); single 128x128 triangular -1e4
    mask applied via an accumulated identity matmul.
  - softmax denominator: DVE bf16 adds -> ones-matmul reduce ->
    reciprocal_approx_fast -> gpsimd partition_broadcast -> one DVE mul.
  - o-proj bias on host; bf16 output partials.
"""

import numpy as np
import ml_dtypes

import concourse.bass as bass
import concourse.mybir as mybir
import concourse.tile as tile
from concourse import bacc

BF16 = ml_dtypes.bfloat16
F32 = mybir.dt.float32
BF = mybir.dt.bfloat16

B, S, H = 2, 2048, 2048
NH, KVH, D = 16, 4, 128
G = NH // KVH  # q heads per kv head / per core
N_CORES = 8
SCALE = 1.0 / float(np.sqrt(D))
NEG = -10000.0

SQ = 512              # q-chunk (psum free width)
NQC = S // SQ         # 4 q chunks
NKT = S // 128        # 16 kv tiles / token tiles
NHT = H // 128        # 16 hidden k-tiles
ROWS = G + 2          # 6 projection row-blocks: 4 q heads, k, v


def build_nc(num_devices: int = N_CORES) -> bass.Bass:
    nc = bacc.Bacc("TRN2", num_devices=num_devices)

    hT = nc.dram_tensor("hT", [NQC, H * SQ], BF, kind="ExternalInput").ap()
    wqkvT = nc.dram_tensor("wqkvT", [ROWS, H * 128], BF, kind="ExternalInput").ap()
    bqkv = nc.dram_tensor("bqkv", [1, ROWS * 128], BF, kind="ExternalInput").ap()
    cosT = nc.dram_tensor("cosT", [128, S], BF, kind="ExternalInput").ap()
    sinTs = nc.dram_tensor("sinTs", [128, S], BF, kind="ExternalInput").ap()
    rotT = nc.dram_tensor("rotT", [128, 128], BF, kind="ExternalInput").ap()
    mask128 = nc.dram_tensor("mask128", [128, 128], BF, kind="ExternalInput").ap()
    woT = nc.dram_tensor("woT", [G * 128, H], BF, kind="ExternalInput").ap()
    id128 = nc.dram_tensor("id128", [128, 128], BF, kind="ExternalInput").ap()
    out = nc.dram_tensor("out", [S, H], BF, kind="ExternalOutput").ap()

    with tile.TileContext(nc) as tc:
        with (
            tc.tile_pool(name="consts", bufs=1) as consts,
            tc.tile_pool(name="persist", bufs=1) as persist,
            tc.tile_pool(name="hp", bufs=2) as hp,
            tc.tile_pool(name="work", bufs=3) as work,
            tc.tile_pool(name="ps", bufs=1, space="PSUM") as ps,
        ):
            # ---- weights: 4 quarter-loads on the sync queue ----
            wq_all = persist.tile([128, NHT * ROWS * 128], BF, tag="wq", name="wq_all")
            for i, m in enumerate([G, G + 1] + list(range(G))):
                nc.sync.dma_start(
                    out=wq_all[:, m * NHT * 128:(m + 1) * NHT * 128],
                    in_=wqkvT[i:i + 1, :].rearrange("o (p w) -> p (o w)", p=128),
                )


            def wq_sl(kt, m):
                return wq_all[:, m * NHT * 128 + kt * 128: m * NHT * 128 + (kt + 1) * 128]

            # ---- constants (scalar queue), bias first ----
            bias_sb = consts.tile([1, ROWS * 128], BF, tag="bias", name="bias")
            nc.scalar.dma_start(out=bias_sb, in_=bqkv)
            cos_sb = consts.tile([128, S], BF, tag="cos", name="cos")
            nc.scalar.dma_start(out=cos_sb, in_=cosT)
            sin_sb = consts.tile([128, S], BF, tag="sin", name="sin")
            nc.scalar.dma_start(out=sin_sb, in_=sinTs)
            mask_sb = consts.tile([128, 128], BF, tag="mask", name="mask")
            nc.scalar.dma_start(out=mask_sb, in_=mask128)
            id_sb = consts.tile([128, 128], BF, tag="id", name="id")
            nc.scalar.dma_start(out=id_sb, in_=id128)
            rt_sb = consts.tile([128, 128], BF, tag="rt", name="rt")
            nc.scalar.dma_start(out=rt_sb, in_=rotT)
            wo_all = persist.tile([128, G * H], BF, tag="wo", name="wo_all")
            ones_col = consts.tile([128, 1], BF, tag="ones_col", name="ones_col")
            nc.vector.memset(ones_col, 1.0)
            ones_row = consts.tile([1, SQ], BF, tag="ones_row", name="ones_row")
            nc.vector.memset(ones_row, 1.0)
            scr_bf = consts.tile([128, SQ], BF, tag="scr", name="scr_bf")
            nc.vector.memset(scr_bf, 0.125)
            warm = ps.tile([128, SQ], F32, tag="small", bufs=1, name="warm")
            for _ in range(16):
                nc.tensor.matmul(warm[0:1, :], ones_col, scr_bf, start=True, stop=True)

            # persistent activations
            qk_sb = [persist.tile([128, S], BF, tag=f"qk{m}", name=f"qk{m}") for m in range(G + 1)]
            vT_sb = persist.tile([128, S], BF, tag="vT", name="vT")
            v_sb = [persist.tile([128, 128], BF, tag=f"v{j}", name=f"v{j}") for j in range(NKT)]
            xT_sb = [persist.tile([128, S], BF, tag=f"xT{h}", name=f"xT{h}") for h in range(G)]
            kT = qk_sb[G]

            dmaq = [nc.sync, nc.gpsimd]
            oproj_ctr = [0]

            def oproj(t):
                """output projection for token tile t."""
                ts_ = slice(t * 128, (t + 1) * 128)
                o_row = work.tile([128, H], BF, tag="o_row", bufs=2, name="o_row")
                for n in range(NQC):
                    ns = slice(n * SQ, (n + 1) * SQ)
                    op = ps.tile([128, SQ], F32, tag="op", bufs=2, name="op")
                    for g in range(G):
                        nc.tensor.matmul(
                            op, xT_sb[g][:, ts_], wo_all[:, g * H + n * SQ: g * H + (n + 1) * SQ],
                            start=(g == 0), stop=(g == G - 1),
                        )
                    if n % 2 == 0:
                        nc.scalar.copy(o_row[:, ns], op)
                    else:
                        nc.vector.tensor_copy(o_row[:, ns], op)
                    if n % 2 == 1:
                        q = dmaq[oproj_ctr[0] % 2]
                        oproj_ctr[0] += 1
                        q.dma_start(
                            out=out[ts_, (n - 1) * SQ:(n + 1) * SQ],
                            in_=o_row[:, (n - 1) * SQ:(n + 1) * SQ],
                        )

            def load_h(c):
                h_all = hp.tile([128, NHT * SQ], BF, tag="hall", name="h_all")
                hc = hT[c:c + 1, :].rearrange("o (p w) -> p (o w)", p=128)
                for qtr in range(8):
                    nc.gpsimd.dma_start(
                        out=h_all[:, qtr * 2 * SQ:(qtr + 1) * 2 * SQ],
                        in_=hc[:, qtr * 2 * SQ:(qtr + 1) * 2 * SQ],
                    )
                return h_all

            h_next = load_h(0)
            for c in range(NQC):
                cs = slice(c * SQ, (c + 1) * SQ)
                h_all = h_next

                # ---- projection + RoPE for chunk c ----
                # row order: k (G), v (G+1) first so attention can start asap
                for m in [G, G + 1] + list(range(G)):
                    mp = ps.tile([128, SQ], F32, tag="mm", bufs=3, name="mp")
                    for kt in range(NHT):
                        nc.tensor.matmul(
                            mp, wq_sl(kt, m),
                            h_all[:, kt * SQ:(kt + 1) * SQ],
                            start=(kt == 0), stop=False,
                        )
                    nc.tensor.matmul(
                        mp, bias_sb[:, m * 128:(m + 1) * 128], ones_row,
                        start=False, stop=True,
                    )
                    if m == G + 1:
                        # v: cast to vT, then produce v blocks for this chunk
                        nc.vector.tensor_copy(vT_sb[:, cs], mp)
                        for jj in range(4 * c, 4 * c + 4):
                            vp = ps.tile([128, SQ], F32, tag="small", bufs=1, name="vp")
                            nc.tensor.matmul(
                                vp[:, :128], vT_sb[:, jj * 128:(jj + 1) * 128],
                                id_sb, start=True, stop=True,
                            )
                            nc.vector.tensor_copy(v_sb[jj], vp[:, :128])
                    else:
                        # q head or k: RoPE
                        tmp = work.tile([128, SQ], BF, tag="tmp", name="tmp")
                        nc.vector.tensor_copy(tmp, mp)
                        rp = ps.tile([128, SQ], F32, tag="small", bufs=1, name="rp")
                        nc.tensor.matmul(rp, rt_sb, tmp, start=True, stop=True)
                        rots = work.tile([128, SQ], BF, tag="rots", name="rots")
                        nc.vector.tensor_mul(rots, rp, sin_sb[:, cs])
                        t1 = work.tile([128, SQ], BF, tag="t1", name="t1")
                        nc.vector.tensor_mul(t1, tmp, cos_sb[:, cs])
                        nc.vector.tensor_add(qk_sb[m][:, cs], t1, rots)

                if c + 1 < NQC:
                    h_next = load_h(c + 1)
                if c == 0:
                    nc.scalar.dma_start(
                        out=wo_all, in_=woT.rearrange("(g p) n -> p g n", p=128)
                    )

                # ---- attention for chunk c (4 heads), interleaving chunk
                # c-1's output projection between heads ----
                njt = 4 * c + 4
                for h in range(G):
                    av = ps.tile([128, SQ], F32, tag="av", bufs=2, name="av")
                    dacc = work.tile([128, SQ], BF, tag="dacc", bufs=2, name="dacc")
                    for j in range(njt):
                        dq = j - 4 * c
                        lo = max(dq, 0) * 128  # first unmasked column
                        sc = ps.tile([128, SQ], F32, tag="mm", bufs=3, name="sc")
                        nc.tensor.matmul(
                            sc[:, lo:], kT[:, j * 128:(j + 1) * 128],
                            qk_sb[h][:, c * SQ + lo:(c + 1) * SQ],
                            start=True, stop=(dq < 0),
                        )
                        if dq >= 0:
                            nc.tensor.matmul(
                                sc[:, lo:lo + 128], id_sb, mask_sb,
                                start=False, stop=True, skip_group_check=True,
                            )
                        ex = work.tile([128, SQ], BF, tag="ex", bufs=3, name="ex")
                        nc.scalar.activation(
                            ex[:, lo:], sc[:, lo:],
                            mybir.ActivationFunctionType.Exp, scale=SCALE,
                        )
                        if j == 0:
                            nc.vector.tensor_copy(dacc, ex)
                        else:
                            nc.vector.tensor_add(
                                dacc[:, lo:], dacc[:, lo:], ex[:, lo:]
                            )
                        nc.tensor.matmul(
                            av[:, lo:], v_sb[j], ex[:, lo:],
                            start=(j == 0), stop=(j == njt - 1),
                            skip_group_check=True,
                        )
                    dn = ps.tile([128, SQ], F32, tag="small", bufs=1, name="dn")
                    nc.tensor.matmul(dn[0:1, :], ones_col, dacc, start=True, stop=True)
                    rd = work.tile([1, SQ], F32, tag="rd", bufs=2, name="rd")
                    nc.vector.reciprocal_approx_fast(rd, dn[0:1, :])
                    rdb = work.tile([1, SQ], BF, tag="rdb", bufs=2, name="rdb")
                    nc.vector.tensor_copy(rdb, rd)
                    bc = ps.tile([128, SQ], F32, tag="small", bufs=1, name="bc")
                    nc.tensor.matmul(bc, ones_row[:, :128], rdb, start=True, stop=True)
                    bcs = work.tile([128, SQ], BF, tag="bcs", bufs=2, name="bcs")
                    nc.scalar.copy(bcs, bc)
                    nc.vector.tensor_mul(xT_sb[h][:, cs], av, bcs)
                    if c > 0:
                        oproj(4 * (c - 1) + h)

            for t in range(12, 16):
                oproj(t)
    nc.compile()
    return nc


def make_in_maps(hidden_states, cos, sin, Wq, bq, Wk, bk, Wv, bv, Wo, bo):
    """Host-side shard/pack. Returns list of 8 input dicts."""
    f32 = np.float32
    cosT = np.ascontiguousarray(np.asarray(cos).T).astype(BF16)
    sinTs = np.ascontiguousarray(np.asarray(sin).T).astype(BF16)
    R = np.zeros((128, 128), f32)
    for d in range(64):
        R[d, d + 64] = -1.0
        R[d + 64, d] = 1.0
    rotT = np.ascontiguousarray(R.T).astype(BF16)
    # triangular -1e4 bias for the in-tile diagonal: unmasked iff q >= kv
    p = np.arange(128)[:, None]
    f = np.arange(128)[None, :]
    mask128 = np.where(f >= p, 0.0, NEG).astype(BF16)
    id128 = np.eye(128, dtype=BF16)

    in_maps = []
    for core in range(N_CORES):
        b, k = core // 4, core % 4
        hTr = np.asarray(hidden_states[b]).T.reshape(NHT, 128, NQC, SQ)
        hT = np.ascontiguousarray(
            hTr.transpose(2, 1, 0, 3).reshape(NQC, 128 * NHT * SQ)
        ).astype(BF16)  # [c][p][kt][q]
        wq = Wq[512 * k:512 * (k + 1)]            # [512, H]
        wk = Wk[128 * k:128 * (k + 1)]            # [128, H]
        wv = Wv[128 * k:128 * (k + 1)]
        wcat = np.concatenate(
            [wk, wv, wq], axis=0
        ).reshape(ROWS, 128, NHT, 128)             # [m'][r][kt][p]
        wqkvT = np.ascontiguousarray(
            wcat.transpose(0, 3, 2, 1).reshape(ROWS, 128 * NHT * 128)
        ).astype(BF16)                             # [m'][p][kt][r]
        bqkv = np.concatenate(
            [bq[512 * k:512 * (k + 1)], bk[128 * k:128 * (k + 1)],
             bv[128 * k:128 * (k + 1)]]
        ).astype(BF16).reshape(1, ROWS * 128)
        woT = np.ascontiguousarray(Wo[:, 512 * k:512 * (k + 1)].T).astype(BF16)
        in_maps.append({
            "hT": hT, "wqkvT": wqkvT, "bqkv": bqkv,
            "cosT": cosT, "sinTs": sinTs, "mask128": mask128, "rotT": rotT,
            "woT": woT, "id128": id128,
        })
    return in_maps


_NC = None


def kernel(**inputs) -> np.ndarray:
    global _NC
    from concourse.bass_utils import run_bass_kernel_spmd

    if _NC is None:
        _NC = build_nc()
    in_maps = make_in_maps(**inputs)
    res = run_bass_kernel_spmd(_NC, in_maps, core_ids=list(range(N_CORES)))
    bo = np.asarray(inputs["bo"], np.float32)
    out = np.zeros((B, S, H), np.float32)
    for core in range(N_CORES):
        out[core // 4] += np.asarray(res.results[core]["out"], np.float32)
    out += bo[None, None, :]
    return out


# revision 11
# speedup vs baseline: 1.0710x; 1.0164x over previous
"""GQA kernel for Trainium2, sharded over 8 NeuronCores.

Problem: B=2, S=2048, H=2048, NH=16 q-heads, KVH=4 kv-heads, D=128.
Sharding: core c -> (batch b = c//4, kv-head k = c%4). Each core computes the
full attention for its 4 query heads + its kv head on its batch, plus the
row-parallel partial of the output projection. Host sums the 4 partials per
batch and adds the output bias.

v3 schedule:
  - chunk-pipelined: per q-chunk c: projection -> attention (4 heads) ->
    output projection, with chunk c-1's o-proj tiles interleaved between
    chunk c's attention blocks so the PE queue always has filler work.
  - batched DMA: weights/hidden loaded as a few big multi-tile transfers,
    split across engine queues (sync=weights/out, gpsimd=hidden, vector=
    consts) so startup loads run in parallel.
  - causal handling: diagonal-block score/exp/AV restricted to the
    unmasked column range (512-128*dq); single 128x128 triangular -1e4
    mask applied via an accumulated identity matmul.
  - softmax denominator: DVE bf16 adds -> ones-matmul reduce ->
    reciprocal_approx_fast -> gpsimd partition_broadcast -> one DVE mul.
  - o-proj bias on host; bf16 output partials.
"""

import numpy as np
import ml_dtypes

import concourse.bass as bass
import concourse.mybir as mybir
import concourse.tile as tile
from concourse import bacc

BF16 = ml_dtypes.bfloat16
F32 = mybir.dt.float32
BF = mybir.dt.bfloat16

B, S, H = 2, 2048, 2048
NH, KVH, D = 16, 4, 128
G = NH // KVH  # q heads per kv head / per core
N_CORES = 8
SCALE = 1.0 / float(np.sqrt(D))
NEG = -10000.0

SQ = 512              # q-chunk (psum free width)
NQC = S // SQ         # 4 q chunks
NKT = S // 128        # 16 kv tiles / token tiles
NHT = H // 128        # 16 hidden k-tiles
ROWS = G + 2          # 6 projection row-blocks: 4 q heads, k, v


def build_nc(num_devices: int = N_CORES) -> bass.Bass:
    nc = bacc.Bacc("TRN2", num_devices=num_devices)

    hT = nc.dram_tensor("hT", [NQC, H * SQ], BF, kind="ExternalInput").ap()
    wqkvT = nc.dram_tensor("wqkvT", [ROWS, H * 128], BF, kind="ExternalInput").ap()
    bqkv = nc.dram_tensor("bqkv", [1, ROWS * 128], BF, kind="ExternalInput").ap()
    cosT = nc.dram_tensor("cosT", [128, S], BF, kind="ExternalInput").ap()
    sinTs = nc.dram_tensor("sinTs", [128, S], BF, kind="ExternalInput").ap()
    rotT = nc.dram_tensor("rotT", [128, 128], BF, kind="ExternalInput").ap()
    mask128 = nc.dram_tensor("mask128", [128, 128], BF, kind="ExternalInput").ap()
    woT = nc.dram_tensor("woT", [G * 128, H], BF, kind="ExternalInput").ap()
    id128 = nc.dram_tensor("id128", [128, 128], BF, kind="ExternalInput").ap()
    out = nc.dram_tensor("out", [S, H], BF, kind="ExternalOutput").ap()

    with tile.TileContext(nc) as tc:
        with (
            tc.tile_pool(name="consts", bufs=1) as consts,
            tc.tile_pool(name="persist", bufs=1) as persist,
            tc.tile_pool(name="hp", bufs=2) as hp,
            tc.tile_pool(name="work", bufs=3) as work,
            tc.tile_pool(name="ps", bufs=1, space="PSUM") as ps,
        ):
            # ---- weights: 4 quarter-loads on the sync queue ----
            wq_all = persist.tile([128, NHT * ROWS * 128], BF, tag="wq", name="wq_all")
            for i, m in enumerate([G, G + 1] + list(range(G))):
                nc.sync.dma_start(
                    out=wq_all[:, m * NHT * 128:(m + 1) * NHT * 128],
                    in_=wqkvT[i:i + 1, :].rearrange("o (p w) -> p (o w)", p=128),
                )


            def wq_sl(kt, m):
                return wq_all[:, m * NHT * 128 + kt * 128: m * NHT * 128 + (kt + 1) * 128]

            # ---- constants (scalar queue), bias first ----
            bias_sb = consts.tile([1, ROWS * 128], BF, tag="bias", name="bias")
            nc.scalar.dma_start(out=bias_sb, in_=bqkv)
            cos_sb = consts.tile([128, S], BF, tag="cos", name="cos")
            nc.scalar.dma_start(out=cos_sb, in_=cosT)
            sin_sb = consts.tile([128, S], BF, tag="sin", name="sin")
            nc.scalar.dma_start(out=sin_sb, in_=sinTs)
            mask_sb = consts.tile([128, 128], BF, tag="mask", name="mask")
            nc.scalar.dma_start(out=mask_sb, in_=mask128)
            id_sb = consts.tile([128, 128], BF, tag="id", name="id")
            nc.scalar.dma_start(out=id_sb, in_=id128)
            rt_sb = consts.tile([128, 128], BF, tag="rt", name="rt")
            nc.scalar.dma_start(out=rt_sb, in_=rotT)
            wo_all = persist.tile([128, G * H], BF, tag="wo", name="wo_all")
            ones_col = consts.tile([128, 1], BF, tag="ones_col", name="ones_col")
            nc.vector.memset(ones_col, 1.0)
            ones_row = consts.tile([1, SQ], BF, tag="ones_row", name="ones_row")
            nc.vector.memset(ones_row, 1.0)


            # persistent activations
            qk_sb = [persist.tile([128, S], BF, tag=f"qk{m}", name=f"qk{m}") for m in range(G + 1)]
            vT_sb = persist.tile([128, S], BF, tag="vT", name="vT")
            v_sb = [persist.tile([128, 128], BF, tag=f"v{j}", name=f"v{j}") for j in range(NKT)]
            xT_sb = [persist.tile([128, S], BF, tag=f"xT{h}", name=f"xT{h}") for h in range(G)]
            kT = qk_sb[G]

            dmaq = [nc.sync, nc.gpsimd]
            oproj_ctr = [0]

            def oproj(t):
                """output projection for token tile t."""
                ts_ = slice(t * 128, (t + 1) * 128)
                o_row = work.tile([128, H], BF, tag="o_row", bufs=2, name="o_row")
                for n in range(NQC):
                    ns = slice(n * SQ, (n + 1) * SQ)
                    op = ps.tile([128, SQ], F32, tag="op", bufs=2, name="op")
                    for g in range(G):
                        nc.tensor.matmul(
                            op, xT_sb[g][:, ts_], wo_all[:, g * H + n * SQ: g * H + (n + 1) * SQ],
                            start=(g == 0), stop=(g == G - 1),
                        )
                    if n % 2 == 0:
                        nc.scalar.copy(o_row[:, ns], op)
                    else:
                        nc.vector.tensor_copy(o_row[:, ns], op)
                    if n % 2 == 1:
                        q = dmaq[oproj_ctr[0] % 2]
                        oproj_ctr[0] += 1
                        q.dma_start(
                            out=out[ts_, (n - 1) * SQ:(n + 1) * SQ],
                            in_=o_row[:, (n - 1) * SQ:(n + 1) * SQ],
                        )

            def load_h(c):
                h_all = hp.tile([128, NHT * SQ], BF, tag="hall", name="h_all")
                hc = hT[c:c + 1, :].rearrange("o (p w) -> p (o w)", p=128)
                for qtr in range(8):
                    nc.gpsimd.dma_start(
                        out=h_all[:, qtr * 2 * SQ:(qtr + 1) * 2 * SQ],
                        in_=hc[:, qtr * 2 * SQ:(qtr + 1) * 2 * SQ],
                    )
                return h_all

            h_next = load_h(0)

            def proj_chunk(c, h_all):
                cs = slice(c * SQ, (c + 1) * SQ)
                for m in [G, G + 1] + list(range(G)):
                    mp = ps.tile([128, SQ], F32, tag="mm", bufs=3, name="mp")
                    for kt in range(NHT):
                        nc.tensor.matmul(
                            mp, wq_sl(kt, m),
                            h_all[:, kt * SQ:(kt + 1) * SQ],
                            start=(kt == 0), stop=False,
                        )
                    nc.tensor.matmul(
                        mp, bias_sb[:, m * 128:(m + 1) * 128], ones_row,
                        start=False, stop=True,
                    )
                    if m == G + 1:
                        # v: cast to vT, then produce v blocks for this chunk
                        nc.vector.tensor_copy(vT_sb[:, cs], mp)
                        for jj in range(4 * c, 4 * c + 4):
                            vp = ps.tile([128, SQ], F32, tag="small", bufs=1, name="vp")
                            nc.tensor.matmul(
                                vp[:, :128], vT_sb[:, jj * 128:(jj + 1) * 128],
                                id_sb, start=True, stop=True,
                            )
                            nc.vector.tensor_copy(v_sb[jj], vp[:, :128])
                    else:
                        # q head or k: RoPE
                        tmp = work.tile([128, SQ], BF, tag="tmp", name="tmp")
                        nc.vector.tensor_copy(tmp, mp)
                        rp = ps.tile([128, SQ], F32, tag="small", bufs=1, name="rp")
                        nc.tensor.matmul(rp, rt_sb, tmp, start=True, stop=True)
                        rots = work.tile([128, SQ], BF, tag="rots", name="rots")
                        nc.vector.tensor_mul(rots, rp, sin_sb[:, cs])
                        t1 = work.tile([128, SQ], BF, tag="t1", name="t1")
                        nc.vector.tensor_mul(t1, tmp, cos_sb[:, cs])
                        nc.vector.tensor_add(qk_sb[m][:, cs], t1, rots)

            def attn_chunk(c):
                cs = slice(c * SQ, (c + 1) * SQ)
                njt = 4 * c + 4
                for h in range(G):
                    av = ps.tile([128, SQ], F32, tag="av", bufs=2, name="av")
                    dacc = work.tile([128, SQ], BF, tag="dacc", bufs=2, name="dacc")
                    for j in range(njt):
                        dq = j - 4 * c
                        lo = max(dq, 0) * 128  # first unmasked column
                        sc = ps.tile([128, SQ], F32, tag="mm", bufs=3, name="sc")
                        nc.tensor.matmul(
                            sc[:, lo:], kT[:, j * 128:(j + 1) * 128],
                            qk_sb[h][:, c * SQ + lo:(c + 1) * SQ],
                            start=True, stop=(dq < 0),
                        )
                        if dq >= 0:
                            nc.tensor.matmul(
                                sc[:, lo:lo + 128], id_sb, mask_sb,
                                start=False, stop=True, skip_group_check=True,
                            )
                        ex = work.tile([128, SQ], BF, tag="ex", bufs=3, name="ex")
                        nc.scalar.activation(
                            ex[:, lo:], sc[:, lo:],
                            mybir.ActivationFunctionType.Exp, scale=SCALE,
                        )
                        if j == 0:
                            nc.vector.tensor_copy(dacc, ex)
                        else:
                            nc.vector.tensor_add(
                                dacc[:, lo:], dacc[:, lo:], ex[:, lo:]
                            )
                        nc.tensor.matmul(
                            av[:, lo:], v_sb[j], ex[:, lo:],
                            start=(j == 0), stop=(j == njt - 1),
                            skip_group_check=True,
                        )
                    dn = ps.tile([128, SQ], F32, tag="small", bufs=1, name="dn")
                    nc.tensor.matmul(dn[0:1, :], ones_col, dacc, start=True, stop=True)
                    rd = work.tile([1, SQ], F32, tag="rd", bufs=2, name="rd")
                    nc.vector.reciprocal_approx_fast(rd, dn[0:1, :])
                    rdb = work.tile([1, SQ], BF, tag="rdb", bufs=2, name="rdb")
                    nc.vector.tensor_copy(rdb, rd)
                    bc = ps.tile([128, SQ], F32, tag="small", bufs=1, name="bc")
                    nc.tensor.matmul(bc, ones_row[:, :128], rdb, start=True, stop=True)
                    bcs = work.tile([128, SQ], BF, tag="bcs", bufs=2, name="bcs")
                    nc.scalar.copy(bcs, bc)
                    nc.vector.tensor_mul(xT_sb[h][:, cs], av, bcs)
                    if c > 0:
                        oproj(4 * (c - 1) + h)

            # software pipeline: proj runs one chunk ahead of attention
            proj_chunk(0, h_next)
            for c in range(NQC):
                if c + 1 < NQC:
                    h2 = load_h(c + 1)
                if c == 0:
                    nc.scalar.dma_start(
                        out=wo_all, in_=woT.rearrange("(g p) n -> p g n", p=128)
                    )
                if c + 1 < NQC:
                    proj_chunk(c + 1, h2)
                attn_chunk(c)
            for t in range(12, 16):
                oproj(t)
    nc.compile()
    return nc


def make_in_maps(hidden_states, cos, sin, Wq, bq, Wk, bk, Wv, bv, Wo, bo):
    """Host-side shard/pack. Returns list of 8 input dicts."""
    f32 = np.float32
    cosT = np.ascontiguousarray(np.asarray(cos).T).astype(BF16)
    sinTs = np.ascontiguousarray(np.asarray(sin).T).astype(BF16)
    R = np.zeros((128, 128), f32)
    for d in range(64):
        R[d, d + 64] = -1.0
        R[d + 64, d] = 1.0
    rotT = np.ascontiguousarray(R.T).astype(BF16)
    # triangular -1e4 bias for the in-tile diagonal: unmasked iff q >= kv
    p = np.arange(128)[:, None]
    f = np.arange(128)[None, :]
    mask128 = np.where(f >= p, 0.0, NEG).astype(BF16)
    id128 = np.eye(128, dtype=BF16)

    in_maps = []
    for core in range(N_CORES):
        b, k = core // 4, core % 4
        hTr = np.asarray(hidden_states[b]).T.reshape(NHT, 128, NQC, SQ)
        hT = np.ascontiguousarray(
            hTr.transpose(2, 1, 0, 3).reshape(NQC, 128 * NHT * SQ)
        ).astype(BF16)  # [c][p][kt][q]
        wq = Wq[512 * k:512 * (k + 1)]            # [512, H]
        wk = Wk[128 * k:128 * (k + 1)]            # [128, H]
        wv = Wv[128 * k:128 * (k + 1)]
        wcat = np.concatenate(
            [wk, wv, wq], axis=0
        ).reshape(ROWS, 128, NHT, 128)             # [m'][r][kt][p]
        wqkvT = np.ascontiguousarray(
            wcat.transpose(0, 3, 2, 1).reshape(ROWS, 128 * NHT * 128)
        ).astype(BF16)                             # [m'][p][kt][r]
        bqkv = np.concatenate(
            [bq[512 * k:512 * (k + 1)], bk[128 * k:128 * (k + 1)],
             bv[128 * k:128 * (k + 1)]]
        ).astype(BF16).reshape(1, ROWS * 128)
        woT = np.ascontiguousarray(Wo[:, 512 * k:512 * (k + 1)].T).astype(BF16)
        in_maps.append({
            "hT": hT, "wqkvT": wqkvT, "bqkv": bqkv,
            "cosT": cosT, "sinTs": sinTs, "mask128": mask128, "rotT": rotT,
            "woT": woT, "id128": id128,
        })
    return in_maps


_NC = None


def kernel(**inputs) -> np.ndarray:
    global _NC
    from concourse.bass_utils import run_bass_kernel_spmd

    if _NC is None:
        _NC = build_nc()
    in_maps = make_in_maps(**inputs)
    res = run_bass_kernel_spmd(_NC, in_maps, core_ids=list(range(N_CORES)))
    bo = np.asarray(inputs["bo"], np.float32)
    out = np.zeros((B, S, H), np.float32)
    for core in range(N_CORES):
        out[core // 4] += np.asarray(res.results[core]["out"], np.float32)
    out += bo[None, None, :]
    return out


# revision 12
# speedup vs baseline: 1.1083x; 1.0348x over previous
"""GQA kernel for Trainium2, sharded over 8 NeuronCores.

Problem: B=2, S=2048, H=2048, NH=16 q-heads, KVH=4 kv-heads, D=128.
Sharding: core c -> (batch b = c//4, kv-head k = c%4). Each core computes the
full attention for its 4 query heads + its kv head on its batch, plus the
row-parallel partial of the output projection. Host sums the 4 partials per
batch and adds the output bias.

v3 schedule:
  - chunk-pipelined: per q-chunk c: projection -> attention (4 heads) ->
    output projection, with chunk c-1's o-proj tiles interleaved between
    chunk c's attention blocks so the PE queue always has filler work.
  - batched DMA: weights/hidden loaded as a few big multi-tile transfers,
    split across engine queues (sync=weights/out, gpsimd=hidden, vector=
    consts) so startup loads run in parallel.
  - causal handling: diagonal-block score/exp/AV restricted to the
    unmasked column range (512-128*dq); single 128x128 triangular -1e4
    mask applied via an accumulated identity matmul.
  - softmax denominator: DVE bf16 adds -> ones-matmul reduce ->
    reciprocal_approx_fast -> gpsimd partition_broadcast -> one DVE mul.
  - o-proj bias on host; bf16 output partials.
"""

import numpy as np
import ml_dtypes

import concourse.bass as bass
import concourse.mybir as mybir
import concourse.tile as tile
from concourse import bacc

BF16 = ml_dtypes.bfloat16
F32 = mybir.dt.float32
BF = mybir.dt.bfloat16

B, S, H = 2, 2048, 2048
NH, KVH, D = 16, 4, 128
G = NH // KVH  # q heads per kv head / per core
N_CORES = 8
SCALE = 1.0 / float(np.sqrt(D))
NEG = -10000.0

SQ = 512              # q-chunk (psum free width)
NQC = S // SQ         # 4 q chunks
NKT = S // 128        # 16 kv tiles / token tiles
NHT = H // 128        # 16 hidden k-tiles
ROWS = G + 2          # 6 projection row-blocks: 4 q heads, k, v


def build_nc(num_devices: int = N_CORES) -> bass.Bass:
    nc = bacc.Bacc("TRN2", num_devices=num_devices)

    hT = nc.dram_tensor("hT", [NQC, H * SQ], BF, kind="ExternalInput").ap()
    wqkvT = nc.dram_tensor("wqkvT", [ROWS, H * 128], BF, kind="ExternalInput").ap()
    bqkv = nc.dram_tensor("bqkv", [128, ROWS], F32, kind="ExternalInput").ap()
    cosT = nc.dram_tensor("cosT", [128, S], BF, kind="ExternalInput").ap()
    sinTs = nc.dram_tensor("sinTs", [128, S], BF, kind="ExternalInput").ap()
    rotT = nc.dram_tensor("rotT", [128, 128], BF, kind="ExternalInput").ap()
    mask128 = nc.dram_tensor("mask128", [128, 128], BF, kind="ExternalInput").ap()
    woT = nc.dram_tensor("woT", [G * 128, H], BF, kind="ExternalInput").ap()
    id128 = nc.dram_tensor("id128", [128, 128], BF, kind="ExternalInput").ap()
    out = nc.dram_tensor("out", [S, H], BF, kind="ExternalOutput").ap()

    with tile.TileContext(nc) as tc:
        with (
            tc.tile_pool(name="consts", bufs=1) as consts,
            tc.tile_pool(name="persist", bufs=1) as persist,
            tc.tile_pool(name="hp", bufs=2) as hp,
            tc.tile_pool(name="work", bufs=3) as work,
            tc.tile_pool(name="ps", bufs=1, space="PSUM") as ps,
        ):
            # ---- weights: 4 quarter-loads on the sync queue ----
            wq_all = persist.tile([128, NHT * ROWS * 128], BF, tag="wq", name="wq_all")
            W = NHT * 128
            for i, m in enumerate([G, G + 1] + list(range(G))):
                wsl = wqkvT[i:i + 1, :].rearrange("o (p w) -> p (o w)", p=128)
                if i == 0:
                    nc.sync.dma_start(
                        out=wq_all[:, m * W:m * W + W // 2], in_=wsl[:, :W // 2]
                    )
                    nc.sync.dma_start(
                        out=wq_all[:, m * W + W // 2:(m + 1) * W], in_=wsl[:, W // 2:]
                    )
                else:
                    nc.sync.dma_start(out=wq_all[:, m * W:(m + 1) * W], in_=wsl)


            def wq_sl(kt, m):
                return wq_all[:, m * NHT * 128 + kt * 128: m * NHT * 128 + (kt + 1) * 128]

            # ---- constants (scalar queue), bias first ----
            bias_sb = consts.tile([128, ROWS], F32, tag="bias", name="bias")
            nc.scalar.dma_start(out=bias_sb, in_=bqkv)
            cos_sb = consts.tile([128, S], BF, tag="cos", name="cos")
            nc.scalar.dma_start(out=cos_sb, in_=cosT)
            sin_sb = consts.tile([128, S], BF, tag="sin", name="sin")
            nc.scalar.dma_start(out=sin_sb, in_=sinTs)
            mask_sb = consts.tile([128, 128], BF, tag="mask", name="mask")
            nc.scalar.dma_start(out=mask_sb, in_=mask128)
            id_sb = consts.tile([128, 128], BF, tag="id", name="id")
            nc.scalar.dma_start(out=id_sb, in_=id128)
            rt_sb = consts.tile([128, 128], BF, tag="rt", name="rt")
            nc.scalar.dma_start(out=rt_sb, in_=rotT)
            wo_all = persist.tile([128, G * H], BF, tag="wo", name="wo_all")
            ones_col = consts.tile([128, 1], BF, tag="ones_col", name="ones_col")
            nc.vector.memset(ones_col, 1.0)
            ones_row = consts.tile([1, SQ], BF, tag="ones_row", name="ones_row")
            nc.vector.memset(ones_row, 1.0)


            # persistent activations
            qk_sb = [persist.tile([128, S], BF, tag=f"qk{m}", name=f"qk{m}") for m in range(G + 1)]
            vT_sb = persist.tile([128, S], BF, tag="vT", name="vT")
            v_sb = [persist.tile([128, 128], BF, tag=f"v{j}", name=f"v{j}") for j in range(NKT)]
            xT_sb = [persist.tile([128, S], BF, tag=f"xT{h}", name=f"xT{h}") for h in range(G)]
            kT = qk_sb[G]

            dmaq = [nc.sync, nc.gpsimd]
            oproj_ctr = [0]

            def oproj(t):
                """output projection for token tile t."""
                ts_ = slice(t * 128, (t + 1) * 128)
                o_row = work.tile([128, H], BF, tag="o_row", bufs=2, name="o_row")
                for n in range(NQC):
                    ns = slice(n * SQ, (n + 1) * SQ)
                    op = ps.tile([128, SQ], F32, tag="op", bufs=2, name="op")
                    for g in range(G):
                        nc.tensor.matmul(
                            op, xT_sb[g][:, ts_], wo_all[:, g * H + n * SQ: g * H + (n + 1) * SQ],
                            start=(g == 0), stop=(g == G - 1),
                        )
                    if n % 2 == 0:
                        nc.scalar.copy(o_row[:, ns], op)
                    else:
                        nc.vector.tensor_copy(o_row[:, ns], op)
                    if t >= 12:
                        q = dmaq[oproj_ctr[0] % 2]
                        oproj_ctr[0] += 1
                        q.dma_start(out=out[ts_, ns], in_=o_row[:, ns])
                    elif n % 2 == 1:
                        q = dmaq[oproj_ctr[0] % 2]
                        oproj_ctr[0] += 1
                        q.dma_start(
                            out=out[ts_, (n - 1) * SQ:(n + 1) * SQ],
                            in_=o_row[:, (n - 1) * SQ:(n + 1) * SQ],
                        )

            def load_h(c):
                h_all = hp.tile([128, NHT * SQ], BF, tag="hall", name="h_all")
                hc = hT[c:c + 1, :].rearrange("o (p w) -> p (o w)", p=128)
                for qtr in range(8):
                    nc.gpsimd.dma_start(
                        out=h_all[:, qtr * 2 * SQ:(qtr + 1) * 2 * SQ],
                        in_=hc[:, qtr * 2 * SQ:(qtr + 1) * 2 * SQ],
                    )
                return h_all

            h_next = load_h(0)

            def proj_chunk(c, h_all):
                cs = slice(c * SQ, (c + 1) * SQ)
                for m in [G, G + 1] + list(range(G)):
                    mp = ps.tile([128, SQ], F32, tag="mm", bufs=3, name="mp")
                    for kt in range(NHT):
                        nc.tensor.matmul(
                            mp, wq_sl(kt, m),
                            h_all[:, kt * SQ:(kt + 1) * SQ],
                            start=(kt == 0), stop=(kt == NHT - 1),
                        )
                    if m == G + 1:
                        # v: bias-add + cast to vT, then v blocks for this chunk
                        nc.scalar.activation(
                            vT_sb[:, cs], mp,
                            mybir.ActivationFunctionType.Identity,
                            bias=bias_sb[:, m:m + 1],
                        )
                        for jj in range(4 * c, 4 * c + 4):
                            vp = ps.tile([128, SQ], F32, tag="small", bufs=1, name="vp")
                            nc.tensor.matmul(
                                vp[:, :128], vT_sb[:, jj * 128:(jj + 1) * 128],
                                id_sb, start=True, stop=True,
                            )
                            nc.vector.tensor_copy(v_sb[jj], vp[:, :128])
                    else:
                        # q head or k: bias-add (ACT) then RoPE
                        tmp = work.tile([128, SQ], BF, tag="tmp", name="tmp")
                        nc.scalar.activation(
                            tmp, mp,
                            mybir.ActivationFunctionType.Identity,
                            bias=bias_sb[:, m:m + 1],
                        )
                        rp = ps.tile([128, SQ], F32, tag="small", bufs=1, name="rp")
                        nc.tensor.matmul(rp, rt_sb, tmp, start=True, stop=True)
                        rots = work.tile([128, SQ], BF, tag="rots", name="rots")
                        nc.vector.tensor_mul(rots, rp, sin_sb[:, cs])
                        t1 = work.tile([128, SQ], BF, tag="t1", name="t1")
                        nc.vector.tensor_mul(t1, tmp, cos_sb[:, cs])
                        nc.vector.tensor_add(qk_sb[m][:, cs], t1, rots)

            def attn_chunk(c):
                cs = slice(c * SQ, (c + 1) * SQ)
                njt = 4 * c + 4
                for h in range(G):
                    av = ps.tile([128, SQ], F32, tag="av", bufs=2, name="av")
                    dacc = work.tile([128, SQ], BF, tag="dacc", bufs=2, name="dacc")
                    for j in range(njt):
                        dq = j - 4 * c
                        lo = max(dq, 0) * 128  # first unmasked column
                        sc = ps.tile([128, SQ], F32, tag="mm", bufs=3, name="sc")
                        nc.tensor.matmul(
                            sc[:, lo:], kT[:, j * 128:(j + 1) * 128],
                            qk_sb[h][:, c * SQ + lo:(c + 1) * SQ],
                            start=True, stop=(dq < 0),
                        )
                        if dq >= 0:
                            nc.tensor.matmul(
                                sc[:, lo:lo + 128], id_sb, mask_sb,
                                start=False, stop=True, skip_group_check=True,
                            )
                        ex = work.tile([128, SQ], BF, tag="ex", bufs=3, name="ex")
                        nc.scalar.activation(
                            ex[:, lo:], sc[:, lo:],
                            mybir.ActivationFunctionType.Exp, scale=SCALE,
                        )
                        if j == 0:
                            nc.vector.tensor_copy(dacc, ex)
                        else:
                            nc.vector.tensor_add(
                                dacc[:, lo:], dacc[:, lo:], ex[:, lo:]
                            )
                        nc.tensor.matmul(
                            av[:, lo:], v_sb[j], ex[:, lo:],
                            start=(j == 0), stop=(j == njt - 1),
                            skip_group_check=True,
                        )
                    dn = ps.tile([128, SQ], F32, tag="small", bufs=1, name="dn")
                    nc.tensor.matmul(dn[0:1, :], ones_col, dacc, start=True, stop=True)
                    rd = work.tile([1, SQ], F32, tag="rd", bufs=2, name="rd")
                    nc.vector.reciprocal_approx_fast(rd, dn[0:1, :])
                    rdb = work.tile([1, SQ], BF, tag="rdb", bufs=2, name="rdb")
                    nc.vector.tensor_copy(rdb, rd)
                    bc = ps.tile([128, SQ], F32, tag="small", bufs=1, name="bc")
                    nc.tensor.matmul(bc, ones_row[:, :128], rdb, start=True, stop=True)
                    bcs = work.tile([128, SQ], BF, tag="bcs", bufs=2, name="bcs")
                    nc.scalar.copy(bcs, bc)
                    nc.vector.tensor_mul(xT_sb[h][:, cs], av, bcs)
                    if c > 0:
                        oproj(4 * (c - 1) + h)

            # software pipeline: proj runs one chunk ahead of attention
            proj_chunk(0, h_next)
            for c in range(NQC):
                if c + 1 < NQC:
                    h2 = load_h(c + 1)
                if c == 0:
                    nc.scalar.dma_start(
                        out=wo_all, in_=woT.rearrange("(g p) n -> p g n", p=128)
                    )
                if c + 1 < NQC:
                    proj_chunk(c + 1, h2)
                attn_chunk(c)
            for t in range(12, 16):
                oproj(t)
    nc.compile()
    return nc


def make_in_maps(hidden_states, cos, sin, Wq, bq, Wk, bk, Wv, bv, Wo, bo):
    """Host-side shard/pack. Returns list of 8 input dicts."""
    f32 = np.float32
    cosT = np.ascontiguousarray(np.asarray(cos).T).astype(BF16)
    sinTs = np.ascontiguousarray(np.asarray(sin).T).astype(BF16)
    R = np.zeros((128, 128), f32)
    for d in range(64):
        R[d, d + 64] = -1.0
        R[d + 64, d] = 1.0
    rotT = np.ascontiguousarray(R.T).astype(BF16)
    # triangular -1e4 bias for the in-tile diagonal: unmasked iff q >= kv
    p = np.arange(128)[:, None]
    f = np.arange(128)[None, :]
    mask128 = np.where(f >= p, 0.0, NEG).astype(BF16)
    id128 = np.eye(128, dtype=BF16)

    in_maps = []
    for core in range(N_CORES):
        b, k = core // 4, core % 4
        hTr = np.asarray(hidden_states[b]).T.reshape(NHT, 128, NQC, SQ)
        hT = np.ascontiguousarray(
            hTr.transpose(2, 1, 0, 3).reshape(NQC, 128 * NHT * SQ)
        ).astype(BF16)  # [c][p][kt][q]
        wq = Wq[512 * k:512 * (k + 1)]            # [512, H]
        wk = Wk[128 * k:128 * (k + 1)]            # [128, H]
        wv = Wv[128 * k:128 * (k + 1)]
        wcat = np.concatenate(
            [wk, wv, wq], axis=0
        ).reshape(ROWS, 128, NHT, 128)             # [m'][r][kt][p]
        wqkvT = np.ascontiguousarray(
            wcat.transpose(0, 3, 2, 1).reshape(ROWS, 128 * NHT * 128)
        ).astype(BF16)                             # [m'][p][kt][r]
        bqkv = np.concatenate(
            [bq[512 * k:512 * (k + 1)], bk[128 * k:128 * (k + 1)],
             bv[128 * k:128 * (k + 1)]]
        ).astype(np.float32).reshape(ROWS, 128).T.copy()  # [128, m] m=q0..3,k,v
        woT = np.ascontiguousarray(Wo[:, 512 * k:512 * (k + 1)].T).astype(BF16)
        in_maps.append({
            "hT": hT, "wqkvT": wqkvT, "bqkv": bqkv,
            "cosT": cosT, "sinTs": sinTs, "mask128": mask128, "rotT": rotT,
            "woT": woT, "id128": id128,
        })
    return in_maps


_NC = None


def kernel(**inputs) -> np.ndarray:
    global _NC
    from concourse.bass_utils import run_bass_kernel_spmd

    if _NC is None:
        _NC = build_nc()
    in_maps = make_in_maps(**inputs)
    res = run_bass_kernel_spmd(_NC, in_maps, core_ids=list(range(N_CORES)))
    bo = np.asarray(inputs["bo"], np.float32)
    out = np.zeros((B, S, H), np.float32)
    for core in range(N_CORES):
        out[core // 4] += np.asarray(res.results[core]["out"], np.float32)
    out += bo[None, None, :]
    return out
